# revision 1
# baseline (speedup 1.0000x reference)
"""Trainium2 Bass kernel for nn_NolinerSEM (90-expert 1D-CNN ensemble).

Mathematical collapse: every conv uses kernel (3,1), stride (2,2), padding
(1,1) on BOTH spatial dims, but the input W dim is 1. After padding (W=3),
the stride-2 width-1 kernel reads padded cols {0, 2} — both zero padding —
so conv1's output is leaky(b1) everywhere and x (and W1) never influence
the output. The remaining network is batch-independent: out[b, i] = c[i],
with (per expert i, leaky slope 0.33, W2k/W3k = conv taps):

  a1 = leaky(b1)                                  [16]
  u0 = leaky((W2k1+W2k2) @ a1 + b2)               [32]  (conv2 h=0 edge)
  u1 = leaky((W2k0+W2k1+W2k2) @ a1 + b2)          [32]  (conv2 h=1..21)
  u2 = leaky((W2k0+W2k1) @ a1 + b2)               [32]  (conv2 h=22 edge)
  e0 = leaky(W3k1 @ u0 + W3k2 @ u1 + b3)          [64]  (conv3 h=0 edge)
  e1 = leaky((W3k0+W3k1+W3k2) @ u1 + b3)          [64]  (conv3 h=1..10)
  e2 = leaky(W3k0 @ u1 + W3k1 @ u2 + b3)          [64]  (conv3 h=11 edge)
  q3 = leaky(b3)                                  [64]  (conv3 w=0 column)
  pool = (12*q3 + e0 + 10*e1 + e2) / 24
  c    = leaky(Wfc . pool + bfc)

Sharding: expert-parallel over 8 cores (12 experts/core, 90 padded to 96
with zero weights). Each core computes its experts' c values and broadcasts
them over the full batch on-device, emitting a [96, 256] shard ((expert,
batch-block) rows); the host folds shards to [2048, 90].

On-core layout:
  stage 2: (expert, conv2-variant) pairs on 36 partitions — one fused
    multiply + segmented-reduce over all 3 u-variants at once.
  stage 3: (expert, c3-block) pairs on 96 partitions (8 c3 per row). The
    PE replicates per-expert u-vectors across blocks via three 0/1
    selector matmuls into PSUM [96, 96] = [u0|u1|u2]. Stage-3 conv taps
    are stored RAW as per-c3 [k0|k1|k2] chunks; the edge matvecs read the
    overlapping windows [k1|k2] (x [u0|u1]) and [k0|k1] (x [u1|u2]), so
    no tap is stored twice. The two 512-element edge multiplies run on
    GpSimd in parallel with the DVE's interior multiply and reduces. A
    final block-diagonal ones matmul group-sums the per-row fc partials
    and replicates the result to all 8 rows of each expert.
"""

from contextlib import ExitStack

import numpy as np

import concourse.bacc as bacc
import concourse.bass as bass
import concourse.mybir as mybir
import concourse.tile as tile
from concourse.bass_utils import run_bass_kernel_spmd

F32 = mybir.dt.float32
Alu = mybir.AluOpType
Ax = mybir.AxisListType

N = 90           # experts
NC = 8           # cores
E = 12           # experts per core (96 padded)
NP = NC * E      # 96
BLK = 8          # c3 blocks per expert in stage 3
CH = 64 // BLK   # c3 channels per (expert, block) row
P3 = E * BLK     # stage-3 partitions (96)
P2 = E * 3       # stage-2 partitions (36)
B = 2048         # batch
BB = B // BLK    # batch block per output row (256)

# in36 column offsets: stage-2 data then the three u-replication selectors
O_B1, O_AF, O_B2, O_SEL = 0, 16, 528, 560
W36 = O_SEL + 3 * P3                          # [36, 848]
# in96 column offsets: interior taps, tail constants, then raw edge taps
O_B1W, O_B3, O_WFC, O_SEL2, O_BFC, O_WRAW = 0, 256, 288, 320, 416, 417
W96 = O_WRAW + CH * 96                        # [96, 1185]


def _emit(tc, d36, d96, d_out):
    nc = tc.nc
    with ExitStack() as ctx:
        pool = ctx.enter_context(tc.tile_pool(name="sb", bufs=1))
        psum = ctx.enter_context(tc.tile_pool(name="ps", bufs=1, space="PSUM"))

        t36 = pool.tile([P2, W36], F32, tag="t36")
        t96 = pool.tile([P3, W96], F32, tag="t96")
        # interleave loads over the two HWDGE engines, consumers-first
        nc.sync.dma_start(t36[:, 0:O_SEL], d36[:, 0:O_SEL])
        nc.scalar.dma_start(t96[:, 0:O_WRAW], d96[:, 0:O_WRAW])
        half = O_WRAW + CH * 48
        nc.sync.dma_start(t96[:, O_WRAW:half], d96[:, O_WRAW:half])
        nc.scalar.dma_start(t96[:, half:W96], d96[:, half:W96])
        nc.sync.dma_start(t36[:, O_SEL:W36], d36[:, O_SEL:W36])

        def leaky(out_ap, in_ap):
            nc.vector.scalar_tensor_tensor(
                out=out_ap, in0=in_ap, scalar=0.33, in1=in_ap,
                op0=Alu.mult, op1=Alu.max)

        # ---- stage 1+2 on [36, *]: uall36[e*3+v, :] = u_v(e)  [36, 32] ----
        a1r = pool.tile([P2, 16], F32, tag="a1r")
        leaky(a1r[:], t36[:, O_B1:O_B1 + 16])

        prod36 = pool.tile([P2, 512], F32, tag="prod36")
        a1b = (a1r[:].rearrange("p (o f) -> p o f", o=1)
               .broadcast_to([P2, 32, 16]))
        nc.vector.tensor_tensor(
            out=prod36[:].rearrange("p (o f) -> p o f", o=32),
            in0=t36[:, O_AF:O_AF + 512].rearrange("p (o f) -> p o f", o=32),
            in1=a1b, op=Alu.mult)
        u36 = pool.tile([P2, 32], F32, tag="u36")
        nc.vector.tensor_reduce(
            out=u36[:], in_=prod36[:].rearrange("p (o f) -> p o f", o=32),
            axis=Ax.X, op=Alu.add)
        ub36 = pool.tile([P2, 32], F32, tag="ub36")
        nc.vector.tensor_add(ub36[:], u36[:], t36[:, O_B2:O_B2 + 32])
        uall36 = pool.tile([P2, 32], F32, tag="uall36")
        leaky(uall36[:], ub36[:])

        # ---- replicate to [96, 96] = [u0 | u1 | u2] per (expert, block) ----
        ps_uop = psum.tile([P3, 96], F32, tag="ps_uop")
        for v in range(3):
            sel = t36[:, O_SEL + v * P3:O_SEL + (v + 1) * P3]
            nc.tensor.matmul(ps_uop[:, v * 32:(v + 1) * 32], sel, uall36[:])
        # SBUF copy so GpSimd (no PSUM access) can read the operands
        uop = pool.tile([P3, 96], F32, tag="uop")
        nc.vector.tensor_copy(uop[:], ps_uop[:])

        # ---- stage 3 ----
        wraw3 = t96[:, O_WRAW:W96].rearrange("p (o f) -> p o f", o=CH)

        def ubc(off, width):
            return (uop[:, off:off + width]
                    .rearrange("p (o f) -> p o f", o=1)
                    .broadcast_to([P3, CH, width]))

        s_all = pool.tile([P3, 24], F32, tag="s_all")
        # edge matvec multiplies on GpSimd (overlapping windows of raw taps)
        m0 = pool.tile([P3, CH * 64], F32, tag="m0")
        nc.gpsimd.tensor_tensor(
            out=m0[:].rearrange("p (o f) -> p o f", o=CH),
            in0=wraw3[:, :, 32:96], in1=ubc(0, 64), op=Alu.mult)
        m2 = pool.tile([P3, CH * 64], F32, tag="m2")
        nc.gpsimd.tensor_tensor(
            out=m2[:].rearrange("p (o f) -> p o f", o=CH),
            in0=wraw3[:, :, 0:64], in1=ubc(32, 64), op=Alu.mult)
        # interior matvec multiply on DVE (host-folded tap sum)
        m1 = pool.tile([P3, CH * 32], F32, tag="m1")
        nc.vector.tensor_tensor(
            out=m1[:].rearrange("p (o f) -> p o f", o=CH),
            in0=t96[:, O_B1W:O_B1W + CH * 32]
                .rearrange("p (o f) -> p o f", o=CH),
            in1=ubc(32, 32), op=Alu.mult)
        nc.vector.tensor_reduce(
            out=s_all[:, 8:16], in_=m1[:].rearrange("p (o f) -> p o f", o=CH),
            axis=Ax.X, op=Alu.add)
        nc.vector.tensor_reduce(
            out=s_all[:, 0:8], in_=m0[:].rearrange("p (o f) -> p o f", o=CH),
            axis=Ax.X, op=Alu.add)
        nc.vector.tensor_reduce(
            out=s_all[:, 16:24], in_=m2[:].rearrange("p (o f) -> p o f", o=CH),
            axis=Ax.X, op=Alu.add)

        # e_all = [leaky(s + b3) | q3 = leaky(b3)]
        e_all = pool.tile([P3, 32], F32, tag="e_all")
        eb = pool.tile([P3, 24], F32, tag="eb")
        nc.vector.tensor_add(eb[:], s_all[:], t96[:, O_B3:O_B3 + 24])
        leaky(e_all[:, 0:24], eb[:])
        leaky(e_all[:, 24:32], t96[:, O_B3 + 24:O_B3 + 32])

        # fc partials with pool weights folded into wfc4
        tt = pool.tile([P3, 32], F32, tag="tt")
        nc.vector.tensor_tensor(out=tt[:], in0=e_all[:],
                                in1=t96[:, O_WFC:O_WFC + 32], op=Alu.mult)
        rowsum = pool.tile([P3, 1], F32, tag="rowsum")
        nc.vector.tensor_reduce(out=rowsum[:], in_=tt[:], axis=Ax.X,
                                op=Alu.add)

        # block-diag ones matmul: group-sum + replicate to all 8 rows
        ps_c = psum.tile([P3, 1], F32, tag="ps_c")
        nc.tensor.matmul(ps_c[:], t96[:, O_SEL2:O_SEL2 + P3], rowsum[:])
        pre = pool.tile([P3, 1], F32, tag="pre")
        nc.vector.tensor_add(pre[:], ps_c[:], t96[:, O_BFC:O_BFC + 1])
        c96 = pool.tile([P3, 1], F32, tag="c96")
        leaky(c96[:], pre[:])

        # broadcast over the batch block and store (split over both DGEs)
        outsb = pool.tile([P3, BB], F32, tag="outsb")
        hb = BB // 2
        nc.vector.tensor_copy(outsb[:, 0:hb], c96[:].broadcast_to([P3, hb]))
        nc.sync.dma_start(d_out[:, 0:hb], outsb[:, 0:hb])
        nc.vector.tensor_copy(outsb[:, hb:BB], c96[:].broadcast_to([P3, hb]))
        nc.scalar.dma_start(d_out[:, hb:BB], outsb[:, hb:BB])


def _build_nc():
    nc = bacc.Bacc("TRN2", target_bir_lowering=False, debug=False,
                   num_devices=NC)
    d36 = nc.dram_tensor("in36", [P2, W36], F32, kind="ExternalInput").ap()
    d96 = nc.dram_tensor("in96", [P3, W96], F32, kind="ExternalInput").ap()
    d_out = nc.dram_tensor("out", [P3, BB], F32, kind="ExternalOutput").ap()
    with tile.TileContext(nc) as tc:
        _emit(tc, d36, d96, d_out)
    nc.compile()
    return nc


def _fold(inputs):
    """Host-side constant folding of the conv taps + per-core sharding."""
    f32 = np.float32
    b1 = np.asarray(inputs["b1"], f32)
    W2 = np.asarray(inputs["W2"], f32)
    b2 = np.asarray(inputs["b2"], f32)
    W3 = np.asarray(inputs["W3"], f32)
    b3 = np.asarray(inputs["b3"], f32)
    Wfc = np.asarray(inputs["Wfc"], f32)
    bfc = np.asarray(inputs["bfc"], f32)

    W2k = W2[..., 0]                       # [90, 32, 16, 3]
    A0 = W2k[..., 1] + W2k[..., 2]
    A1 = W2k[..., 0] + W2k[..., 1] + W2k[..., 2]
    A2 = W2k[..., 0] + W2k[..., 1]
    af36 = np.stack([A0.reshape(N, 512), A1.reshape(N, 512),
                     A2.reshape(N, 512)], axis=1)          # [90, 3, 512]
    b1r36 = np.repeat(b1[:, None, :], 3, axis=1)           # [90, 3, 16]
    b2r36 = np.repeat(b2[:, None, :], 3, axis=1)           # [90, 3, 32]

    W3k = W3[..., 0]                       # [90, 64, 32, 3]
    # raw taps per c3 as [k0|k1|k2]; edge matvecs read overlapping windows
    wraw = np.ascontiguousarray(W3k.transpose(0, 1, 3, 2)) \
        .reshape(N, BLK, CH * 96)                          # [90, BLK, 768]
    B1 = W3k[..., 0] + W3k[..., 1] + W3k[..., 2]           # [90, 64, 32]
    b1w = B1.reshape(N, BLK, CH * 32)
    b3p4 = np.tile(b3.reshape(N, BLK, CH), (1, 1, 4))      # [90, BLK, 32]
    wfcp = (Wfc[:, 0, :] / f32(24.0)).reshape(N, BLK, CH)
    wfc4 = np.concatenate([wfcp, f32(10.0) * wfcp, wfcp,
                           f32(12.0) * wfcp], axis=-1)     # [90, BLK, 32]
    bfc96 = np.repeat(bfc, BLK, axis=0).reshape(N, BLK, 1)

    def pad(a):
        return np.concatenate(
            [a, np.zeros((NP - N,) + a.shape[1:], f32)], axis=0)

    af36, b1r36, b2r36, wraw, b1w, b3p4, wfc4, bfc96 = (
        pad(af36), pad(b1r36), pad(b2r36), pad(wraw), pad(b1w), pad(b3p4),
        pad(wfc4), pad(bfc96))

    # selectors (identical on every core)
    sel3 = np.zeros((3, P2, P3), f32)      # sel3[v, e*3+w, e*8+blk]
    for e in range(E):
        for v in range(3):
            sel3[v, e * 3 + v, e * BLK:(e + 1) * BLK] = 1.0
    sel2 = np.kron(np.eye(E, dtype=f32), np.ones((BLK, BLK), f32))  # [96,96]

    in_maps = []
    for k in range(NC):
        sl = slice(k * E, (k + 1) * E)
        in36 = np.concatenate([
            b1r36[sl].reshape(P2, 16),
            af36[sl].reshape(P2, 512),
            b2r36[sl].reshape(P2, 32),
            sel3[0], sel3[1], sel3[2],
        ], axis=1)
        in96 = np.concatenate([
            b1w[sl].reshape(P3, CH * 32),
            b3p4[sl].reshape(P3, 32),
            wfc4[sl].reshape(P3, 32),
            sel2,
            bfc96[sl].reshape(P3, 1),
            wraw[sl].reshape(P3, CH * 96),
        ], axis=1)
        in_maps.append({"in36": np.ascontiguousarray(in36),
                        "in96": np.ascontiguousarray(in96)})
    return in_maps


def _assemble(shards):
    """[96, 256] per core -> [2048, 90] full output."""
    full = np.concatenate(
        [s.reshape(E, BLK, BB).reshape(E, B) for s in shards], axis=0)
    return np.ascontiguousarray(full[:N].T.astype(np.float32))


_NC_CACHE = []


def _reset_device():
    """Best-effort recovery of a wedged axon terminal."""
    try:
        import ctypes
        ctypes.CDLL("/opt/axon/libaxon_pjrt.so").axon_reset()
    except Exception:
        pass


def _run(inputs, trace=False, **kwargs):
    in_maps = _fold(inputs)
    if not _NC_CACHE:
        _NC_CACHE.append(_build_nc())
    nc = _NC_CACHE[0]
    try:
        res = run_bass_kernel_spmd(nc, in_maps, core_ids=list(range(NC)),
                                   trace=trace, **kwargs)
    except Exception:
        _reset_device()
        res = run_bass_kernel_spmd(nc, in_maps, core_ids=list(range(NC)),
                                   trace=trace, **kwargs)
    out = _assemble([res.results[k]["out"] for k in range(NC)])
    return out, res


def kernel(**inputs) -> np.ndarray:
    out, _ = _run(inputs, trace=False)
    return out



# revision 9
# speedup vs baseline: 1.6439x; 1.6439x over previous
"""Trainium2 Bass kernel for nn_NolinerSEM (90-expert 1D-CNN ensemble).

Mathematical collapse: every conv uses kernel (3,1), stride (2,2), padding
(1,1) on BOTH spatial dims, but the input W dim is 1. After padding (W=3),
the stride-2 width-1 kernel reads padded cols {0, 2} — both zero padding —
so conv1's output is leaky(b1) everywhere and x (and W1) never influence
the output. The remaining network is batch-independent: out[b, i] = c[i],
with (per expert i, leaky slope 0.33, W2k/W3k = conv taps):

  a1 = leaky(b1)                                  [16]
  u0 = leaky((W2k1+W2k2) @ a1 + b2)               [32]  (conv2 h=0 edge)
  u1 = leaky((W2k0+W2k1+W2k2) @ a1 + b2)          [32]  (conv2 h=1..21)
  u2 = leaky((W2k0+W2k1) @ a1 + b2)               [32]  (conv2 h=22 edge)
  e0 = leaky(W3k1 @ u0 + W3k2 @ u1 + b3)          [64]  (conv3 h=0 edge)
  e1 = leaky((W3k0+W3k1+W3k2) @ u1 + b3)          [64]  (conv3 h=1..10)
  e2 = leaky(W3k0 @ u1 + W3k1 @ u2 + b3)          [64]  (conv3 h=11 edge)
  q3 = leaky(b3)                                  [64]  (conv3 w=0 column)
  pool = (12*q3 + e0 + 10*e1 + e2) / 24
  c    = leaky(Wfc . pool + bfc)

Implementation: everything runs TRANSPOSED (experts along the free dim,
feature lanes along partitions) so every matvec maps onto the PE as a tiny
per-expert matmul with the expert's weights as the stationary operand and
the expert's activation vector as a 1-column moving operand. The Act engine
applies every leaky (Lrelu, alpha=0.33) straight out of PSUM; DVE only
seeds a ones-lane. All weights ship as fp16 (PE fp16 matmuls are 4x faster,
DMA bytes halve; rel-err stays ~1e-3, well inside the 2e-2 gate).

PE tiling rules couple operand base partitions: stationary/moving bases
must match and lie on the quadrant grid (<=32 rows: 0/32/64/96; <=64 rows:
0/64; else 0). Hence uT rows = [u0 | u1 | u2 | ones] at 0/32/64/96, e2
splits into two accumulating matmuls (k0T x u1 at base 32, [k1T;b3] x
[u2|1] at base 64), e0/int biases arrive via one identity-stationary
matmul, and bfc via a one-cell-stationary matmul into ps3.

Per-core operands (12 experts, e = local expert index):
  in17 [17,1164] -> t17 @ rows 0:17:  b1T+one | A_v(e).T + b2-lane [17x32]
  in65 [65,1536] -> t97 @ rows 32:97: B1(e).T (u1 band) | e2 packed
                                      (k0T u1-band; k1T u2-band; b3 lane)
  in64 [64,917]  -> t64 @ rows 0:64:  e0 [k1T;k2T] | identity | b3 (e0,int)
                                      pairs | wfc*scale/24 x4 | b3T | bfcT
                                      | one cell
  out  [1,12] f32: c values; host broadcasts to [2048, 90].
"""

from contextlib import ExitStack

import numpy as np

import concourse.bacc as bacc
import concourse.bass as bass
import concourse.mybir as mybir
import concourse.tile as tile
from concourse.bass_utils import run_bass_kernel_spmd

F32 = mybir.dt.float32
F16 = mybir.dt.float16
ALPHA = 0.33
LRELU = mybir.ActivationFunctionType.Prelu   # Lrelu's table bakes alpha=0.01

N = 90           # experts
NC = 8           # cores
E = 12           # experts per core (96 padded)
NP = NC * E      # 96

# in17 columns
A_OFF = 12                 # A-variant blocks
W17 = A_OFF + 36 * 32      # 1164
# in65 columns (lands at SBUF partitions 32:97)
E2_OFF = E * 64            # 768: packed e2 blocks after the int blocks
W65 = 2 * E * 64           # 1536
# in64 columns
ID_OFF = E * 64            # 768: identity
B3P_OFF = ID_OFF + 64      # 832: b3 (e0,int) pairs
FC_OFF = B3P_OFF + 2 * E   # 856: fc stationaries (4 per expert)
B3T_OFF = FC_OFF + 4 * E   # 904: b3T for q3
BFC_OFF = B3T_OFF + E      # 916: bfcT row + the 1.0 cell
W64 = BFC_OFF + 13         # 929


def _emit(tc, d17, d65, d64, d_out):
    nc = tc.nc
    with ExitStack() as ctx:
        pool = ctx.enter_context(tc.tile_pool(name="sb", bufs=1))
        psum = ctx.enter_context(tc.tile_pool(name="ps", bufs=1, space="PSUM"))

        t17 = pool.tile([17, W17], F16, tag="t17")
        t97 = pool.tile([97, W65], F16, tag="t97")
        t64 = pool.tile([64, W64], F16, tag="t64")
        nc.sync.dma_start(t17[:], d17)
        nc.sync.dma_start(t97[32:97, :], d65)
        nc.sync.dma_start(t64[:], d64)

        pt = psum.tile([97, 12], F32, tag="pt")
        ps1 = psum.tile([64, 36], F32, tag="ps1")
        ps3 = psum.tile([1, 12], F32, tag="ps3")

        def leaky(out_ap, in_ap):
            nc.scalar.activation(out_ap, in_ap, LRELU, alpha=ALPHA)

        nc.vector.memset(pt[96:97, :], 1.0)   # ones-lane of uT

        # stage 1: a1 = leaky(b1), transposed [17, e]; row 16 = leaky(1) = 1
        a1T = pool.tile([17, 12], F16, tag="a1T")
        leaky(a1T[:], t17[0:17, 0:12])

        # stage 2 on PE: pt[(v), e] = A_v(e) @ a1(e) + b2(e)
        for e in range(E):
            for v in range(3):
                a_cols = A_OFF + (e * 3 + v) * 32
                nc.tensor.matmul(pt[32 * v:32 * v + 32, e:e + 1],
                                 t17[0:17, a_cols:a_cols + 32],
                                 a1T[0:17, e:e + 1])
        uT = pool.tile([97, 12], F16, tag="uT")
        leaky(uT[:], pt[:])          # rows [u0|u1|u2|1] per expert column

        # q3 = leaky(b3) for the fc stage (DVE: Act stays on the uT->eT path)
        q3T = pool.tile([64, 12], F16, tag="q3T")
        nc.vector.scalar_tensor_tensor(
            out=q3T[:], in0=t64[0:64, B3T_OFF:B3T_OFF + E], scalar=ALPHA,
            in1=t64[0:64, B3T_OFF:B3T_OFF + E],
            op0=mybir.AluOpType.mult, op1=mybir.AluOpType.max)

        # stage 3 on PE: ps1[c3, (e,var)] = conv3 pre-activations.
        # PSUM accumulation groups must be CONSECUTIVE matmuls (an
        # intervening start=True discards the open group), so each column's
        # start..stop pair is adjacent; e0/int biases come from an
        # identity-stationary matmul moving the expert's b3 column.
        ident = t64[0:64, ID_OFF:ID_OFF + 64]
        for e in range(E):
            i_cols = 64 * e
            nc.tensor.matmul(ps1[:, 3 * e + 1:3 * e + 2],
                             t97[32:64, i_cols:i_cols + 64],
                             uT[32:64, e:e + 1],
                             start=True, stop=False, skip_group_check=True)
            nc.tensor.matmul(ps1[:, 3 * e + 1:3 * e + 2], ident,
                             t64[0:64, B3P_OFF + 2 * e + 1:B3P_OFF + 2 * e + 2],
                             start=False, stop=True, skip_group_check=True)
            e2_cols = E2_OFF + 64 * e
            nc.tensor.matmul(ps1[:, 3 * e + 2:3 * e + 3],
                             t97[32:64, e2_cols:e2_cols + 64],
                             uT[32:64, e:e + 1],
                             start=True, stop=False, skip_group_check=True)
            nc.tensor.matmul(ps1[:, 3 * e + 2:3 * e + 3],
                             t97[64:97, e2_cols:e2_cols + 64],
                             uT[64:97, e:e + 1],
                             start=False, stop=True, skip_group_check=True)
        for e in range(E):
            nc.tensor.matmul(ps1[:, 3 * e:3 * e + 1],
                             t64[0:64, 64 * e:64 * e + 64],
                             uT[0:64, e:e + 1],
                             start=True, stop=False, skip_group_check=True)
            nc.tensor.matmul(ps1[:, 3 * e:3 * e + 1], ident,
                             t64[0:64, B3P_OFF + 2 * e:B3P_OFF + 2 * e + 1],
                             start=False, stop=True, skip_group_check=True)
        eT = pool.tile([64, 36], F16, tag="eT")
        leaky(eT[:], ps1[:])

        # fc on PE: per expert, 4 accumulating matmuls (e0/e1/e2/q3 terms,
        # pool scale folded in) closed by a one-cell bfc matmul
        one_cell = t64[0:1, BFC_OFF + 12:BFC_OFF + 13]
        for e in range(E):
            movs = (eT[0:64, 3 * e:3 * e + 1],
                    eT[0:64, 3 * e + 1:3 * e + 2],
                    eT[0:64, 3 * e + 2:3 * e + 3],
                    q3T[0:64, e:e + 1])
            for j, mov in enumerate(movs):
                nc.tensor.matmul(ps3[0:1, e:e + 1],
                                 t64[0:64, FC_OFF + 4 * e + j:FC_OFF + 4 * e + j + 1],
                                 mov, start=(j == 0), stop=False,
                                 skip_group_check=True)
            nc.tensor.matmul(ps3[0:1, e:e + 1],
                             t64[0:1, BFC_OFF + e:BFC_OFF + e + 1],
                             one_cell, start=False, stop=True,
                             skip_group_check=True)

        outc = pool.tile([1, 12], F32, tag="outc")
        leaky(outc[:], ps3[:])
        nc.sync.dma_start(d_out, outc[:])


def _build_nc():
    nc = bacc.Bacc("TRN2", target_bir_lowering=False, debug=False,
                   num_devices=NC)
    d17 = nc.dram_tensor("in17", [17, W17], F16, kind="ExternalInput").ap()
    d65 = nc.dram_tensor("in65", [65, W65], F16, kind="ExternalInput").ap()
    d64 = nc.dram_tensor("in64", [64, W64], F16, kind="ExternalInput").ap()
    d_out = nc.dram_tensor("out", [1, 12], F32, kind="ExternalOutput").ap()
    with tile.TileContext(nc) as tc:
        _emit(tc, d17, d65, d64, d_out)
    nc.compile()
    return nc


def _fold(inputs):
    """Host-side linear preprocessing of the conv weights + per-core shard."""
    f32, f16 = np.float32, np.float16
    b1 = np.asarray(inputs["b1"], f32)
    W2k = np.asarray(inputs["W2"], f32)[..., 0]    # [90, 32, 16, 3]
    b2 = np.asarray(inputs["b2"], f32)
    W3k = np.asarray(inputs["W3"], f32)[..., 0]    # [90, 64, 32, 3]
    b3 = np.asarray(inputs["b3"], f32)
    Wfc = np.asarray(inputs["Wfc"], f32)[:, 0, :]  # [90, 64]
    bfc = np.asarray(inputs["bfc"], f32)[:, 0]     # [90]

    A = np.stack([W2k[..., 1] + W2k[..., 2],
                  W2k[..., 0] + W2k[..., 1] + W2k[..., 2],
                  W2k[..., 0] + W2k[..., 1]], axis=1)   # [90, 3, 32, 16]
    B1 = W3k.sum(-1)                                    # [90, 64, 32]

    def pad(a):
        return np.concatenate(
            [a, np.zeros((NP - N,) + a.shape[1:], a.dtype)], axis=0)

    A, B1, W3k, b1, b2, b3, Wfc, bfc = (
        pad(A), pad(B1), pad(W3k), pad(b1), pad(b2), pad(b3), pad(Wfc),
        pad(bfc))

    fc_scale = np.array([1.0, 10.0, 1.0, 12.0], f32) / 24.0

    in_maps = []
    for k in range(NC):
        sl = slice(k * E, (k + 1) * E)
        Ak, B1k, W3kk = A[sl], B1[sl], W3k[sl]
        b1k, b2k, b3k = b1[sl], b2[sl], b3[sl]
        Wfck, bfck = Wfc[sl], bfc[sl]

        d17 = np.zeros((17, W17), f32)
        d17[0:16, 0:12] = b1k.T
        d17[16, 0:12] = 1.0
        for e in range(E):
            for v in range(3):
                c = A_OFF + (e * 3 + v) * 32
                d17[0:16, c:c + 32] = Ak[e, v].T       # [16, 32]
                d17[16, c:c + 32] = b2k[e]

        d65 = np.zeros((65, W65), f32)
        for e in range(E):
            c = 64 * e
            d65[0:32, c:c + 64] = B1k[e].T             # int (x u1)
            c = E2_OFF + 64 * e
            d65[0:32, c:c + 64] = W3kk[e, :, :, 0].T   # e2: k0 (x u1)
            d65[32:64, c:c + 64] = W3kk[e, :, :, 1].T  # e2: k1 (x u2)
            d65[64, c:c + 64] = b3k[e]                 # e2: b3 (x 1)

        d64 = np.zeros((64, W64), f32)
        for e in range(E):
            c = 64 * e
            d64[0:32, c:c + 64] = W3kk[e, :, :, 1].T   # e0: k1 (x u0)
            d64[32:64, c:c + 64] = W3kk[e, :, :, 2].T  # e0: k2 (x u1)
            d64[0:64, B3P_OFF + 2 * e] = b3k[e]        # e0 bias column
            d64[0:64, B3P_OFF + 2 * e + 1] = b3k[e]    # int bias column
            for j in range(4):
                d64[0:64, FC_OFF + 4 * e + j] = Wfck[e] * fc_scale[j]
            d64[0:64, B3T_OFF + e] = b3k[e]
        d64[0:64, ID_OFF:ID_OFF + 64] = np.eye(64, dtype=f32)
        d64[0, BFC_OFF:BFC_OFF + 12] = bfck
        d64[0, BFC_OFF + 12] = 1.0

        in_maps.append({"in17": np.ascontiguousarray(d17, dtype=f16),
                        "in65": np.ascontiguousarray(d65, dtype=f16),
                        "in64": np.ascontiguousarray(d64, dtype=f16)})
    return in_maps


def _assemble(shards):
    """8 x [1, 12] expert constants -> full [2048, 90] output."""
    c = np.concatenate([s[0] for s in shards])[:N].astype(np.float32)
    return np.ascontiguousarray(np.broadcast_to(c, (2048, N)))


_NC_CACHE = []


def _reset_device():
    """Best-effort recovery of a wedged axon terminal."""
    try:
        import ctypes
        ctypes.CDLL("/opt/axon/libaxon_pjrt.so").axon_reset()
    except Exception:
        pass


def _run(inputs, trace=False, **kwargs):
    in_maps = _fold(inputs)
    if not _NC_CACHE:
        _NC_CACHE.append(_build_nc())
    nc = _NC_CACHE[0]
    try:
        res = run_bass_kernel_spmd(nc, in_maps, core_ids=list(range(NC)),
                                   trace=trace, **kwargs)
    except Exception:
        _reset_device()
        res = run_bass_kernel_spmd(nc, in_maps, core_ids=list(range(NC)),
                                   trace=trace, **kwargs)
    out = _assemble([res.results[k]["out"] for k in range(NC)])
    return out, res


def kernel(**inputs) -> np.ndarray:
    out, _ = _run(inputs, trace=False)
    return out


# revision 13
# speedup vs baseline: 1.7052x; 1.0373x over previous
"""Trainium2 Bass kernel for nn_NolinerSEM (90-expert 1D-CNN ensemble).

Mathematical collapse: every conv uses kernel (3,1), stride (2,2), padding
(1,1) on BOTH spatial dims, but the input W dim is 1. After padding (W=3),
the stride-2 width-1 kernel reads padded cols {0, 2} — both zero padding —
so conv1's output is leaky(b1) everywhere and x (and W1) never influence
the output. The remaining network is batch-independent: out[b, i] = c[i],
with (per expert i, leaky slope 0.33, W2k/W3k = conv taps):

  a1 = leaky(b1)                                  [16]
  u0 = leaky((W2k1+W2k2) @ a1 + b2)               [32]  (conv2 h=0 edge)
  u1 = leaky((W2k0+W2k1+W2k2) @ a1 + b2)          [32]  (conv2 h=1..21)
  u2 = leaky((W2k0+W2k1) @ a1 + b2)               [32]  (conv2 h=22 edge)
  e0 = leaky(W3k1 @ u0 + W3k2 @ u1 + b3)          [64]  (conv3 h=0 edge)
  e1 = leaky((W3k0+W3k1+W3k2) @ u1 + b3)          [64]  (conv3 h=1..10)
  e2 = leaky(W3k0 @ u1 + W3k1 @ u2 + b3)          [64]  (conv3 h=11 edge)
  q3 = leaky(b3)                                  [64]  (conv3 w=0 column)
  pool = (12*q3 + e0 + 10*e1 + e2) / 24
  c    = leaky(Wfc . pool + bfc)

Implementation: everything runs TRANSPOSED (experts along the free dim,
feature lanes along partitions) so every matvec maps onto the PE as a tiny
per-expert matmul with the expert's weights as the stationary operand and
the expert's activation vector as a 1-column moving operand. The Act engine
applies every leaky (Lrelu, alpha=0.33) straight out of PSUM; DVE only
seeds a ones-lane. All weights ship as fp16 (PE fp16 matmuls are 4x faster,
DMA bytes halve; rel-err stays ~1e-3, well inside the 2e-2 gate).

PE tiling rules couple operand base partitions: stationary/moving bases
must match and lie on the quadrant grid (<=32 rows: 0/32/64/96; <=64 rows:
0/64; else 0). Hence uT rows = [u0 | u1 | u2 | ones] at 0/32/64/96, e2
splits into two accumulating matmuls (k0T x u1 at base 32, [k1T;b3] x
[u2|1] at base 64), e0/int biases arrive via one identity-stationary
matmul, and bfc via a one-cell-stationary matmul into ps3.

Per-core operands (12 experts, e = local expert index):
  in17 [17,1164] -> t17 @ rows 0:17:  b1T+one | A_v(e).T + b2-lane [17x32]
  in65 [65,1536] -> t97 @ rows 32:97: B1(e).T (u1 band) | e2 packed
                                      (k0T u1-band; k1T u2-band; b3 lane)
  in64 [64,917]  -> t64 @ rows 0:64:  e0 [k1T;k2T] | identity | b3 (e0,int)
                                      pairs | wfc*scale/24 x4 | b3T | bfcT
                                      | one cell
  out  [1,12] f32: c values; host broadcasts to [2048, 90].
"""

from contextlib import ExitStack

import numpy as np

import concourse.bacc as bacc
import concourse.bass as bass
import concourse.mybir as mybir
import concourse.tile as tile
from concourse.bass_utils import run_bass_kernel_spmd

F32 = mybir.dt.float32
F16 = mybir.dt.float16
ALPHA = 0.33
LRELU = mybir.ActivationFunctionType.Prelu   # Lrelu's table bakes alpha=0.01

N = 90           # experts
NC = 8           # cores
E = 12           # experts per core (96 padded)
NP = NC * E      # 96

# in17 columns
A_OFF = 12                 # A-variant blocks
W17 = A_OFF + 36 * 32      # 1164
# in65 columns (lands at SBUF partitions 32:97); the int blocks' dram row 64
# carries b3 so [zeros;b3] x [u2|1] closes the e0/int bias for free
E2_OFF = E * 64            # 768: packed e2 blocks after the int blocks
W65 = 2 * E * 64           # 1536
# in64 columns
FC_OFF = E * 64            # 768: fc stationaries (4 per expert)
B3T_OFF = FC_OFF + 4 * E   # 816: b3T for q3
BFC_OFF = B3T_OFF + E      # 828: bfcT row + the 1.0 cell
W64 = BFC_OFF + 13         # 841


def _emit(tc, d17, d65, d64, d_out):
    nc = tc.nc
    with ExitStack() as ctx:
        pool = ctx.enter_context(tc.tile_pool(name="sb", bufs=1))
        psum = ctx.enter_context(tc.tile_pool(name="ps", bufs=1, space="PSUM"))

        t17 = pool.tile([17, W17], F16, tag="t17")
        t97 = pool.tile([97, W65], F16, tag="t97")
        t64 = pool.tile([64, W64], F16, tag="t64")
        # t64 goes through the Pool engine's SWDGE: its descriptor gen runs
        # concurrently with the HWDGE pipe, so its transfer slots in between
        # t17's and t97's on the shared DMA engines
        nc.sync.dma_start(t17[:], d17)
        nc.sync.dma_start(t97[32:97, :], d65)
        nc.gpsimd.dma_start(t64[:], d64)

        pt = psum.tile([97, 12], F32, tag="pt")
        ps1 = psum.tile([64, 36], F32, tag="ps1")
        ps3 = psum.tile([1, 12], F32, tag="ps3")

        def leaky(out_ap, in_ap):
            nc.scalar.activation(out_ap, in_ap, LRELU, alpha=ALPHA)

        nc.vector.memset(pt[96:97, :], 1.0)   # ones-lane of uT

        # stage 1: a1 = leaky(b1), transposed [17, e]; row 16 = leaky(1) = 1
        a1T = pool.tile([17, 12], F16, tag="a1T")
        leaky(a1T[:], t17[0:17, 0:12])

        # stage 2 on PE: pt[(v), e] = A_v(e) @ a1(e) + b2(e)
        for e in range(E):
            for v in range(3):
                a_cols = A_OFF + (e * 3 + v) * 32
                nc.tensor.matmul(pt[32 * v:32 * v + 32, e:e + 1],
                                 t17[0:17, a_cols:a_cols + 32],
                                 a1T[0:17, e:e + 1])
        uT = pool.tile([97, 12], F16, tag="uT")
        leaky(uT[:], pt[:])          # rows [u0|u1|u2|1] per expert column

        # q3 = leaky(b3) for the fc stage (DVE: Act stays on the uT->eT path)
        q3T = pool.tile([64, 12], F16, tag="q3T")
        nc.vector.scalar_tensor_tensor(
            out=q3T[:], in0=t64[0:64, B3T_OFF:B3T_OFF + E], scalar=ALPHA,
            in1=t64[0:64, B3T_OFF:B3T_OFF + E],
            op0=mybir.AluOpType.mult, op1=mybir.AluOpType.max)

        # stage 3 on PE: ps1[c3, (e,var)] = conv3 pre-activations.
        # PSUM accumulation groups must be CONSECUTIVE matmuls (an
        # intervening start=True discards the open group). The int blocks'
        # [zeros;b3] band (partitions 64:97) times [u2|1] adds exactly b3,
        # closing both the int and e0 columns. int/e2 pairs run first (they
        # only need the t97 DMA); the e0 pairs follow.
        for e in range(E):
            i_cols = 64 * e
            b3st = t97[64:97, i_cols:i_cols + 64]
            nc.tensor.matmul(ps1[:, 3 * e + 1:3 * e + 2],
                             t97[32:64, i_cols:i_cols + 64],
                             uT[32:64, e:e + 1],
                             start=True, stop=False, skip_group_check=True)
            nc.tensor.matmul(ps1[:, 3 * e + 1:3 * e + 2], b3st,
                             uT[64:97, e:e + 1],
                             start=False, stop=True, skip_group_check=True)
            e2_cols = E2_OFF + 64 * e
            nc.tensor.matmul(ps1[:, 3 * e + 2:3 * e + 3],
                             t97[32:64, e2_cols:e2_cols + 64],
                             uT[32:64, e:e + 1],
                             start=True, stop=False, skip_group_check=True)
            nc.tensor.matmul(ps1[:, 3 * e + 2:3 * e + 3],
                             t97[64:97, e2_cols:e2_cols + 64],
                             uT[64:97, e:e + 1],
                             start=False, stop=True, skip_group_check=True)
        for e in range(E):
            b3st = t97[64:97, 64 * e:64 * e + 64]
            nc.tensor.matmul(ps1[:, 3 * e:3 * e + 1],
                             t64[0:64, 64 * e:64 * e + 64],
                             uT[0:64, e:e + 1],
                             start=True, stop=False, skip_group_check=True)
            nc.tensor.matmul(ps1[:, 3 * e:3 * e + 1], b3st,
                             uT[64:97, e:e + 1],
                             start=False, stop=True, skip_group_check=True)
        eT = pool.tile([64, 36], F16, tag="eT")
        leaky(eT[:], ps1[:])

        # fc on PE: per expert, 4 accumulating matmuls (e0/e1/e2/q3 terms,
        # pool scale folded in) closed by a one-cell bfc matmul
        one_cell = t64[0:1, BFC_OFF + 12:BFC_OFF + 13]
        for e in range(E):
            movs = (eT[0:64, 3 * e:3 * e + 1],
                    eT[0:64, 3 * e + 1:3 * e + 2],
                    eT[0:64, 3 * e + 2:3 * e + 3],
                    q3T[0:64, e:e + 1])
            for j, mov in enumerate(movs):
                nc.tensor.matmul(ps3[0:1, e:e + 1],
                                 t64[0:64, FC_OFF + 4 * e + j:FC_OFF + 4 * e + j + 1],
                                 mov, start=(j == 0), stop=False,
                                 skip_group_check=True)
            nc.tensor.matmul(ps3[0:1, e:e + 1],
                             t64[0:1, BFC_OFF + e:BFC_OFF + e + 1],
                             one_cell, start=False, stop=True,
                             skip_group_check=True)

        outc = pool.tile([1, 12], F32, tag="outc")
        leaky(outc[:], ps3[:])
        nc.sync.dma_start(d_out, outc[:])


def _build_nc():
    nc = bacc.Bacc("TRN2", target_bir_lowering=False, debug=False,
                   num_devices=NC)
    d17 = nc.dram_tensor("in17", [17, W17], F16, kind="ExternalInput").ap()
    d65 = nc.dram_tensor("in65", [65, W65], F16, kind="ExternalInput").ap()
    d64 = nc.dram_tensor("in64", [64, W64], F16, kind="ExternalInput").ap()
    d_out = nc.dram_tensor("out", [1, 12], F32, kind="ExternalOutput").ap()
    with tile.TileContext(nc) as tc:
        _emit(tc, d17, d65, d64, d_out)
    nc.compile()
    return nc


def _fold(inputs):
    """Host-side linear preprocessing of the conv weights + per-core shard."""
    f32, f16 = np.float32, np.float16
    b1 = np.asarray(inputs["b1"], f32)
    W2k = np.asarray(inputs["W2"], f32)[..., 0]    # [90, 32, 16, 3]
    b2 = np.asarray(inputs["b2"], f32)
    W3k = np.asarray(inputs["W3"], f32)[..., 0]    # [90, 64, 32, 3]
    b3 = np.asarray(inputs["b3"], f32)
    Wfc = np.asarray(inputs["Wfc"], f32)[:, 0, :]  # [90, 64]
    bfc = np.asarray(inputs["bfc"], f32)[:, 0]     # [90]

    A = np.stack([W2k[..., 1] + W2k[..., 2],
                  W2k[..., 0] + W2k[..., 1] + W2k[..., 2],
                  W2k[..., 0] + W2k[..., 1]], axis=1)   # [90, 3, 32, 16]
    B1 = W3k.sum(-1)                                    # [90, 64, 32]

    def pad(a):
        return np.concatenate(
            [a, np.zeros((NP - N,) + a.shape[1:], a.dtype)], axis=0)

    A, B1, W3k, b1, b2, b3, Wfc, bfc = (
        pad(A), pad(B1), pad(W3k), pad(b1), pad(b2), pad(b3), pad(Wfc),
        pad(bfc))

    fc_scale = np.array([1.0, 10.0, 1.0, 12.0], f32) / 24.0

    in_maps = []
    for k in range(NC):
        sl = slice(k * E, (k + 1) * E)
        Ak, B1k, W3kk = A[sl], B1[sl], W3k[sl]
        b1k, b2k, b3k = b1[sl], b2[sl], b3[sl]
        Wfck, bfck = Wfc[sl], bfc[sl]

        d17 = np.zeros((17, W17), f32)
        d17[0:16, 0:12] = b1k.T
        d17[16, 0:12] = 1.0
        for e in range(E):
            for v in range(3):
                c = A_OFF + (e * 3 + v) * 32
                d17[0:16, c:c + 32] = Ak[e, v].T       # [16, 32]
                d17[16, c:c + 32] = b2k[e]

        d65 = np.zeros((65, W65), f32)
        for e in range(E):
            c = 64 * e
            d65[0:32, c:c + 64] = B1k[e].T             # int (x u1)
            d65[64, c:c + 64] = b3k[e]                 # [0;b3] bias band
            c = E2_OFF + 64 * e
            d65[0:32, c:c + 64] = W3kk[e, :, :, 0].T   # e2: k0 (x u1)
            d65[32:64, c:c + 64] = W3kk[e, :, :, 1].T  # e2: k1 (x u2)
            d65[64, c:c + 64] = b3k[e]                 # e2: b3 (x 1)

        d64 = np.zeros((64, W64), f32)
        for e in range(E):
            c = 64 * e
            d64[0:32, c:c + 64] = W3kk[e, :, :, 1].T   # e0: k1 (x u0)
            d64[32:64, c:c + 64] = W3kk[e, :, :, 2].T  # e0: k2 (x u1)
            for j in range(4):
                d64[0:64, FC_OFF + 4 * e + j] = Wfck[e] * fc_scale[j]
            d64[0:64, B3T_OFF + e] = b3k[e]
        d64[0, BFC_OFF:BFC_OFF + 12] = bfck
        d64[0, BFC_OFF + 12] = 1.0

        in_maps.append({"in17": np.ascontiguousarray(d17, dtype=f16),
                        "in65": np.ascontiguousarray(d65, dtype=f16),
                        "in64": np.ascontiguousarray(d64, dtype=f16)})
    return in_maps


def _assemble(shards):
    """8 x [1, 12] expert constants -> full [2048, 90] output."""
    c = np.concatenate([s[0] for s in shards])[:N].astype(np.float32)
    return np.ascontiguousarray(np.broadcast_to(c, (2048, N)))


_NC_CACHE = []


def _reset_device():
    """Best-effort recovery of a wedged axon terminal."""
    try:
        import ctypes
        ctypes.CDLL("/opt/axon/libaxon_pjrt.so").axon_reset()
    except Exception:
        pass


def _run(inputs, trace=False, **kwargs):
    in_maps = _fold(inputs)
    if not _NC_CACHE:
        _NC_CACHE.append(_build_nc())
    nc = _NC_CACHE[0]
    try:
        res = run_bass_kernel_spmd(nc, in_maps, core_ids=list(range(NC)),
                                   trace=trace, **kwargs)
    except Exception:
        _reset_device()
        res = run_bass_kernel_spmd(nc, in_maps, core_ids=list(range(NC)),
                                   trace=trace, **kwargs)
    out = _assemble([res.results[k]["out"] for k in range(NC)])
    return out, res


def kernel(**inputs) -> np.ndarray:
    out, _ = _run(inputs, trace=False)
    return out
